# revision 7
# baseline (speedup 1.0000x reference)
"""Trainium2 Bass kernel for nn_MetaBaseline (global-cosine + DN4 few-shot scoring).

Math (per episode b):
  global: logits[q,k] = <qmean_hat, bmean_hat>          (means over the 5x5 spatial grid)
  DN4:    sim[q,p,k,l] = <q_patch[q,p], s_col_hat[k,l]>  -> sum of top-neighbor_k over l,
          summed over p, / neighbor_k
  out = r0 * logits + r1 * dn4

Device strategy (data-parallel, 8 episodes per NeuronCore):
  - host pre-normalizes the support side and appends the 5 class-mean columns:
    s_ext [640, 130] per episode; host lays query out as q_mat [640, 1920] (qp-major,
    zero-padded from 1875), both cast to bf16.
  - PE: sim_ext[qp, 0:130] = q_mat^T @ s_ext as 15 qp-tiles x 5 k-tiles of
    [128,128]x[128,130] bf16 matmuls accumulating in fp32 PSUM.
  - ACT copies PSUM -> SBUF (bf16).
  - DVE Max8 gives the top-8 of each 25-value support-patch group in one op;
    reduce_sum of the first neighbor_k gives the per-(patch,class) DN4 term.
  - per-patch 1/||q_patch|| scaling (ACT, per-partition scale), then a tiny
    matmul against a constant 0/1 patch->query aggregation matrix contracts the
    125 patches of each query across partitions. The raw class-mean projections
    (cols 125:130) go through the same aggregation unscaled.
  - host applies 1/(25*||q_mean||), neighbor_k, and the r-weighted combine.
"""
import numpy as np
import ml_dtypes

N_CORES = 8
B, WAY, SHOT, D, H, W = 64, 5, 1, 640, 5, 5
NQ = 75
HW = H * W                 # 25
QP = NQ * HW               # 1875 query patches per episode
NT = 15                    # qp tiles of 128
QP_PAD = NT * 128          # 1920
ND = D // 128              # 5 contraction tiles
EPC = B // N_CORES         # 8 episodes per core
SCOLS = WAY * HW + WAY     # 130
GEPS = 1e-12               # eps of the global-cosine branch (torch F.normalize)

_CACHE = {}
_DEFER_TAIL = True


def _build(k: int):
    """Build + compile the SPMD NEFF for top-k = k (k <= 8)."""
    import concourse.bacc as bacc
    import concourse.mybir as mybir
    import concourse.tile as tile

    bf16 = mybir.dt.bfloat16
    f32 = mybir.dt.float32
    COPY = mybir.ActivationFunctionType.Copy

    nc = bacc.Bacc("TRN2", target_bir_lowering=False, debug=False)
    qm = nc.dram_tensor("qm", [EPC, ND, 128, QP_PAD], bf16, kind="ExternalInput")
    se = nc.dram_tensor("se", [EPC, ND, 128, SCOLS], bf16, kind="ExternalInput")
    amat = nc.dram_tensor("amat", [128, NT * NQ], bf16, kind="ExternalInput")
    invq = nc.dram_tensor("invq", [128, EPC * NT], f32, kind="ExternalInput")
    out = nc.dram_tensor("out", [EPC, WAY, 2 * NQ], f32, kind="ExternalOutput")

    with tile.TileContext(nc) as tc:
        with (
            tc.tile_pool(name="const", bufs=1) as cpool,
            tc.tile_pool(name="q", bufs=2 * ND) as qpool,
            tc.tile_pool(name="s", bufs=2 * ND) as spool,
            tc.tile_pool(name="simps", bufs=3, space="PSUM") as simpool,
            tc.tile_pool(name="acc", bufs=2, space="PSUM") as accpool,
            tc.tile_pool(name="simsb", bufs=4) as sbpool,
            tc.tile_pool(name="out8", bufs=2) as o8pool,
            tc.tile_pool(name="draw", bufs=2) as drpool,
            tc.tile_pool(name="dsc", bufs=4) as dscpool,
            tc.tile_pool(name="osb", bufs=2) as opool,
        ):
            amat_t = cpool.tile([128, NT * NQ], bf16)
            nc.sync.dma_start(amat_t[:], amat[:])
            invq_t = cpool.tile([128, EPC * NT], f32)
            nc.sync.dma_start(invq_t[:], invq[:])

            pending = []  # deferred per-episode tail: (e, out8, dn4_ps, glob_ps)

            def emit_tail():
                if not pending:
                    return
                e, out8, dn4_ps, glob_ps = pending.pop()
                # sum of the top-k values of each (qp, way) group
                draw = drpool.tile([128, NT * WAY], f32)
                o8v = out8[:].rearrange("p (g e) -> p g e", e=8)[:, :, 0:k]
                nc.vector.reduce_sum(draw[:], o8v, axis=mybir.AxisListType.X)
                for t in range(NT):
                    dsc = dscpool.tile([128, WAY], bf16)
                    col = e * NT + t
                    nc.scalar.activation(
                        dsc[:], draw[:, t * WAY:(t + 1) * WAY], COPY,
                        scale=invq_t[:, col:col + 1],
                    )
                    nc.tensor.matmul(
                        dn4_ps[:], dsc[:], amat_t[:, t * NQ:(t + 1) * NQ],
                        start=(t == 0), stop=(t == NT - 1),
                    )
                osb = opool.tile([WAY, 2 * NQ], f32)
                nc.scalar.activation(osb[:, 0:NQ], dn4_ps[:], COPY)
                nc.scalar.activation(osb[:, NQ:2 * NQ], glob_ps[:], COPY)
                nc.sync.dma_start(out[e], osb[:])

            for e in range(EPC):
                qts = []
                sts = []
                for d in range(ND):
                    qt = qpool.tile([128, QP_PAD], bf16)
                    nc.sync.dma_start(qt[:], qm[e, d])
                    qts.append(qt)
                    st = spool.tile([128, SCOLS], bf16)
                    nc.sync.dma_start(st[:], se[e, d])
                    sts.append(st)
                dn4_ps = accpool.tile([WAY, NQ], f32, tag="dn4ps")
                glob_ps = accpool.tile([WAY, NQ], f32, tag="globps")
                out8 = o8pool.tile([128, NT * WAY * 8], bf16)
                for t in range(NT):
                    simps = simpool.tile([128, SCOLS], f32)
                    for d in range(ND):
                        nc.tensor.matmul(
                            simps[:], qts[d][:, t * 128:(t + 1) * 128], sts[d][:],
                            start=(d == 0), stop=(d == ND - 1),
                        )
                    simsb = sbpool.tile([128, SCOLS], bf16)
                    nc.scalar.activation(simsb[:], simps[:], COPY)
                    for kk in range(WAY):
                        g = t * WAY + kk
                        nc.vector.max(
                            out8[:, g * 8:(g + 1) * 8],
                            simsb[:, kk * HW:(kk + 1) * HW],
                        )
                    nc.tensor.matmul(
                        glob_ps[:], simsb[:, WAY * HW:SCOLS],
                        amat_t[:, t * NQ:(t + 1) * NQ],
                        start=(t == 0), stop=(t == NT - 1),
                    )
                    if t == 2 and _DEFER_TAIL:
                        emit_tail()  # previous episode's reduction work
                pending.append((e, out8, dn4_ps, glob_ps))
                if not _DEFER_TAIL:
                    emit_tail()
            emit_tail()
    nc.compile()
    return nc


def kernel(base, query, r, neighbor_k):
    from concourse.bass_utils import run_bass_kernel_spmd

    k = int(neighbor_k)
    assert 1 <= k <= 8, f"top-k must fit the Max8 output, got {k}"
    base = np.asarray(base, dtype=np.float32).reshape(B, WAY, D, HW)
    query = np.asarray(query, dtype=np.float32).reshape(B, NQ, D, HW)
    r = np.asarray(r, dtype=np.float32)

    # ---- host prep (layout + normalization metadata) ----
    # support: normalized columns + normalized class means -> s_ext [B, D, 130]
    s_norm = base / np.linalg.norm(base, axis=2, keepdims=True)
    bmean = base.mean(axis=3)                                     # [B, way, D]
    bm = bmean / np.maximum(
        np.linalg.norm(bmean, axis=2, keepdims=True), GEPS)
    s_ext = np.empty((B, D, SCOLS), dtype=np.float32)
    s_ext[:, :, :WAY * HW] = s_norm.transpose(0, 2, 1, 3).reshape(B, D, WAY * HW)
    s_ext[:, :, WAY * HW:] = bm.transpose(0, 2, 1)
    s_ext = s_ext.reshape(B, ND, 128, SCOLS).astype(ml_dtypes.bfloat16)

    # query: q_mat [B, D, 1920] (qp-major, zero-padded), bf16
    q_mat = np.zeros((B, D, QP_PAD), dtype=ml_dtypes.bfloat16)
    q_mat[:, :, :QP] = query.transpose(0, 2, 1, 3).reshape(B, D, QP)
    q_mat = q_mat.reshape(B, ND, 128, QP_PAD)

    # per-patch inverse norms, arranged [128, EPC*NT] per core
    qn = np.sqrt(np.einsum("bqdp,bqdp->bqp", query, query, dtype=np.float64))
    invq_full = np.zeros((B, QP_PAD), dtype=np.float32)
    invq_full[:, :QP] = (1.0 / qn).reshape(B, QP).astype(np.float32)
    invq_full = invq_full.reshape(B, NT, 128)

    # query-mean norms for the global branch
    qmean = query.mean(axis=3)                                    # [B, nq, D]
    qmn = np.maximum(np.linalg.norm(qmean, axis=2), GEPS)         # [B, nq]

    # patch->query aggregation matrix (0/1), [128, NT*NQ]
    am = np.zeros((128, NT, NQ), dtype=ml_dtypes.bfloat16)
    for t in range(NT):
        qp_idx = t * 128 + np.arange(128)
        valid = qp_idx < QP
        am[valid, t, qp_idx[valid] // HW] = 1.0
    am = am.reshape(128, NT * NQ)

    if k not in _CACHE:
        _CACHE[k] = _build(k)
    nc = _CACHE[k]

    in_maps = []
    for c in range(N_CORES):
        sl = slice(c * EPC, (c + 1) * EPC)
        in_maps.append({
            "qm": np.ascontiguousarray(q_mat[sl]),
            "se": np.ascontiguousarray(s_ext[sl]),
            "amat": am,
            "invq": np.ascontiguousarray(
                invq_full[sl].transpose(2, 0, 1).reshape(128, EPC * NT)),
        })
    global _LAST_IN_MAPS
    _LAST_IN_MAPS = in_maps
    res = run_bass_kernel_spmd(nc, in_maps, list(range(N_CORES)))
    dev = np.stack([res.results[c]["out"] for c in range(N_CORES)])  # [C, EPC, WAY, 150]
    dev = dev.reshape(B, WAY, 2 * NQ)

    dn4 = dev[:, :, :NQ].transpose(0, 2, 1) / k                   # [B, nq, way]
    glob = dev[:, :, NQ:].transpose(0, 2, 1) / (HW * qmn[:, :, None])
    return (r[0] * glob + r[1] * dn4).astype(np.float32)


# revision 8
# speedup vs baseline: 1.2718x; 1.2718x over previous
"""Trainium2 Bass kernel for nn_MetaBaseline (global-cosine + DN4 few-shot scoring).

Math (per episode b):
  global: logits[q,k] = <qmean_hat, bmean_hat>          (means over the 5x5 spatial grid)
  DN4:    sim[q,p,k,l] = <q_patch[q,p], s_col_hat[k,l]>  -> sum of top-neighbor_k over l,
          summed over p, / neighbor_k
  out = r0 * logits + r1 * dn4

Device strategy (data-parallel, 8 episodes per NeuronCore):
  - host pre-normalizes the support side and appends the 5 class-mean columns:
    s_ext [640, 130] per episode; query laid out as q_mat [640, 1920] (qp-major,
    zero-padded from 1875); both bf16.
  - PE: sim_ext[qp, 0:130] = q_mat^T @ s_ext as 15 qp-tiles x 5 k-tiles of
    [128,128]x[128,130] bf16 matmuls accumulating in fp32 PSUM.
  - ACT copies PSUM -> SBUF (bf16) with a fused per-partition 1/||q_patch|| scale
    (row-positive, so DN4's top-k selection is unaffected).
  - DVE Max8 gives the top-8 of each 25-value support-patch group in one op;
    one strided reduce_sum of the first neighbor_k per episode gives the
    per-(patch,class) DN4 terms.
  - tiny matmuls against patch->query aggregation matrices contract the 25
    patches of each query across partitions: the DN4 terms against a constant
    0/1 matrix, the class-mean projections (cols 125:130, which carry a spurious
    1/||q_patch|| factor) against a host-built A*||q_patch|| matrix that undoes it.
  - host applies 1/(25*||q_mean||), neighbor_k, and the r-weighted combine.
"""
import numpy as np
import ml_dtypes

N_CORES = 8
B, WAY, SHOT, D, H, W = 64, 5, 1, 640, 5, 5
NQ = 75
HW = H * W                 # 25
QP = NQ * HW               # 1875 query patches per episode
NT = 15                    # qp tiles of 128
QP_PAD = NT * 128          # 1920
ND = D // 128              # 5 contraction tiles
EPC = B // N_CORES         # 8 episodes per core
SCOLS = WAY * HW + WAY     # 130
GEPS = 1e-12               # eps of the global-cosine branch (torch F.normalize)

_CACHE = {}
_LAST_IN_MAPS = None


def _build(k: int):
    """Build + compile the SPMD NEFF for top-k = k (k <= 8)."""
    import concourse.bacc as bacc
    import concourse.mybir as mybir
    import concourse.tile as tile

    bf16 = mybir.dt.bfloat16
    f32 = mybir.dt.float32
    COPY = mybir.ActivationFunctionType.Copy

    nc = bacc.Bacc("TRN2", target_bir_lowering=False, debug=False)
    qm = nc.dram_tensor("qm", [EPC, ND, 128, QP_PAD], bf16, kind="ExternalInput")
    se = nc.dram_tensor("se", [EPC, ND, 128, SCOLS], bf16, kind="ExternalInput")
    amat = nc.dram_tensor("amat", [128, NT * NQ], bf16, kind="ExternalInput")
    am2 = nc.dram_tensor("am2", [EPC, 128, NT * NQ], bf16, kind="ExternalInput")
    invq = nc.dram_tensor("invq", [128, EPC * NT], f32, kind="ExternalInput")
    out = nc.dram_tensor("out", [EPC, WAY, 2 * NQ], f32, kind="ExternalOutput")

    with tile.TileContext(nc) as tc:
        with (
            tc.tile_pool(name="const", bufs=1) as cpool,
            tc.tile_pool(name="q", bufs=2 * ND) as qpool,
            tc.tile_pool(name="s", bufs=2 * ND) as spool,
            tc.tile_pool(name="a2", bufs=2) as a2pool,
            tc.tile_pool(name="simps", bufs=4, space="PSUM") as simpool,
            tc.tile_pool(name="acc", bufs=2, space="PSUM") as accpool,
            tc.tile_pool(name="simsb", bufs=NT + 3) as sbpool,
            tc.tile_pool(name="out8", bufs=2) as o8pool,
            tc.tile_pool(name="draw", bufs=2) as drpool,
            tc.tile_pool(name="osb", bufs=2) as opool,
        ):
            amat_t = cpool.tile([128, NT * NQ], bf16)
            nc.sync.dma_start(amat_t[:], amat[:])
            invq_t = cpool.tile([128, EPC * NT], f32)
            nc.sync.dma_start(invq_t[:], invq[:])

            pending = []  # deferred tail: (e, draw, dn4_ps, glob_ps)

            def emit_tail():
                if not pending:
                    return
                e, draw, dn4_ps, glob_ps = pending.pop()
                for t in range(NT):
                    nc.tensor.matmul(
                        dn4_ps[:], draw[:, t * WAY:(t + 1) * WAY],
                        amat_t[:, t * NQ:(t + 1) * NQ],
                        start=(t == 0), stop=(t == NT - 1),
                    )
                osb = opool.tile([WAY, 2 * NQ], f32)
                nc.scalar.activation(osb[:, 0:NQ], dn4_ps[:], COPY)
                nc.scalar.activation(osb[:, NQ:2 * NQ], glob_ps[:], COPY)
                nc.sync.dma_start(out[e], osb[:])

            for e in range(EPC):
                qts = []
                sts = []
                for d in range(ND):
                    qt = qpool.tile([128, QP_PAD], bf16)
                    nc.sync.dma_start(qt[:], qm[e, d])
                    qts.append(qt)
                    st = spool.tile([128, SCOLS], bf16)
                    nc.sync.dma_start(st[:], se[e, d])
                    sts.append(st)
                a2t = a2pool.tile([128, NT * NQ], bf16)
                nc.sync.dma_start(a2t[:], am2[e])
                dn4_ps = accpool.tile([WAY, NQ], f32, tag="dn4ps")
                glob_ps = accpool.tile([WAY, NQ], f32, tag="globps")
                out8 = o8pool.tile([128, NT * WAY * 8], bf16)
                simsbs = []
                for t in range(NT):
                    simps = simpool.tile([128, SCOLS], f32)
                    for d in range(ND):
                        nc.tensor.matmul(
                            simps[:], qts[d][:, t * 128:(t + 1) * 128], sts[d][:],
                            start=(d == 0), stop=(d == ND - 1),
                        )
                    simsb = sbpool.tile([128, SCOLS], bf16)
                    nc.scalar.activation(
                        simsb[:], simps[:], COPY,
                        scale=invq_t[:, e * NT + t:e * NT + t + 1],
                    )
                    simsbs.append(simsb)
                    for kk in range(WAY):
                        g = t * WAY + kk
                        nc.vector.max(
                            out8[:, g * 8:(g + 1) * 8],
                            simsb[:, kk * HW:(kk + 1) * HW],
                        )
                    if t == 2:
                        emit_tail()  # previous episode's dn4 aggregation
                for t in range(NT):
                    nc.tensor.matmul(
                        glob_ps[:], simsbs[t][:, WAY * HW:SCOLS],
                        a2t[:, t * NQ:(t + 1) * NQ],
                        start=(t == 0), stop=(t == NT - 1),
                    )
                draw = drpool.tile([128, NT * WAY], bf16)
                o8v = out8[:].rearrange("p (g e) -> p g e", e=8)[:, :, 0:k]
                with nc.allow_low_precision("bf16 top-k sums feed a bf16 matmul"):
                    nc.vector.reduce_sum(draw[:], o8v, axis=mybir.AxisListType.X)
                pending.append((e, draw, dn4_ps, glob_ps))
            emit_tail()
    nc.compile()
    return nc


def kernel(base, query, r, neighbor_k):
    from concourse.bass_utils import run_bass_kernel_spmd

    k = int(neighbor_k)
    assert 1 <= k <= 8, f"top-k must fit the Max8 output, got {k}"
    base = np.asarray(base, dtype=np.float32).reshape(B, WAY, D, HW)
    query = np.asarray(query, dtype=np.float32).reshape(B, NQ, D, HW)
    r = np.asarray(r, dtype=np.float32)

    # ---- host prep (layout + normalization metadata) ----
    # support: normalized columns + normalized class means -> s_ext [B, D, 130]
    s_norm = base / np.linalg.norm(base, axis=2, keepdims=True)
    bmean = base.mean(axis=3)                                     # [B, way, D]
    bm = bmean / np.maximum(
        np.linalg.norm(bmean, axis=2, keepdims=True), GEPS)
    s_ext = np.empty((B, D, SCOLS), dtype=np.float32)
    s_ext[:, :, :WAY * HW] = s_norm.transpose(0, 2, 1, 3).reshape(B, D, WAY * HW)
    s_ext[:, :, WAY * HW:] = bm.transpose(0, 2, 1)
    s_ext = s_ext.reshape(B, ND, 128, SCOLS).astype(ml_dtypes.bfloat16)

    # query: q_mat [B, D, 1920] (qp-major, zero-padded), bf16
    q_mat = np.zeros((B, D, QP_PAD), dtype=ml_dtypes.bfloat16)
    q_mat[:, :, :QP] = query.transpose(0, 2, 1, 3).reshape(B, D, QP)
    q_mat = q_mat.reshape(B, ND, 128, QP_PAD)

    # per-patch norms; inverse arranged [128, EPC*NT] per core
    qn = np.sqrt(np.einsum("bqdp,bqdp->bqp", query, query)).reshape(B, QP)
    qn_pad = np.zeros((B, QP_PAD), dtype=np.float32)
    qn_pad[:, :QP] = qn
    invq_full = np.zeros((B, QP_PAD), dtype=np.float32)
    invq_full[:, :QP] = 1.0 / qn
    invq_full = invq_full.reshape(B, NT, 128)

    # query-mean norms for the global branch
    qmean = query.mean(axis=3)                                    # [B, nq, D]
    qmn = np.maximum(np.linalg.norm(qmean, axis=2), GEPS)         # [B, nq]

    # patch->query aggregation matrix (0/1), [128, NT*NQ]; and A*||q_patch||
    am = np.zeros((128, NT, NQ), dtype=np.float32)
    for t in range(NT):
        qp_idx = t * 128 + np.arange(128)
        valid = qp_idx < QP
        am[valid, t, qp_idx[valid] // HW] = 1.0
    am2 = am[None] * qn_pad.reshape(B, NT, 128).transpose(0, 2, 1)[:, :, :, None]
    am = am.reshape(128, NT * NQ).astype(ml_dtypes.bfloat16)
    am2 = am2.reshape(B, 128, NT * NQ).astype(ml_dtypes.bfloat16)

    if k not in _CACHE:
        _CACHE[k] = _build(k)
    nc = _CACHE[k]

    in_maps = []
    for c in range(N_CORES):
        sl = slice(c * EPC, (c + 1) * EPC)
        in_maps.append({
            "qm": np.ascontiguousarray(q_mat[sl]),
            "se": np.ascontiguousarray(s_ext[sl]),
            "amat": am,
            "am2": np.ascontiguousarray(am2[sl]),
            "invq": np.ascontiguousarray(
                invq_full[sl].transpose(2, 0, 1).reshape(128, EPC * NT)),
        })
    global _LAST_IN_MAPS
    _LAST_IN_MAPS = in_maps
    res = run_bass_kernel_spmd(nc, in_maps, list(range(N_CORES)))
    dev = np.stack([res.results[c]["out"] for c in range(N_CORES)])  # [C, EPC, WAY, 150]
    dev = dev.reshape(B, WAY, 2 * NQ)

    dn4 = dev[:, :, :NQ].transpose(0, 2, 1) / k                   # [B, nq, way]
    glob = dev[:, :, NQ:].transpose(0, 2, 1) / (HW * qmn[:, :, None])
    return (r[0] * glob + r[1] * dn4).astype(np.float32)
